# revision 29
# baseline (speedup 1.0000x reference)
"""Gated TCN layer (fully conditioned) as a Bass/Tile kernel on 8 NeuronCores.

Reference computation (per sample b):
    kern = (c @ adapter_w + adapter_b).reshape(2*CH, CH, K)
    y    = dilated causal conv of x with per-sample kern (K=3, dil=4)
    y   += (c @ bias_w + bias_b)[:, None]
    z    = tanh(y[:CH]) * sigmoid(y[CH:])
    out  = resi_w @ z + resi_b + x
Returns (out, z).

Sharding: data-parallel over batch, 2 samples per core. Both samples are
stacked on the 128 SBUF partitions (rows 0-63 = sample 0, 64-127 = sample 1)
with block-diagonal per-tap dynamic kernels, so every matmul contracts over
128 partitions and every activation / vector op runs at full 128-partition
width. All heavy traffic and matmuls are bf16 (outputs are converted back to
f32 on the host); PSUM accumulation stays f32.
"""

import numpy as np

from concourse import bacc, mybir, tile
from concourse.bass_utils import run_bass_kernel_spmd

K = 3
DIL = 4
CH = 64
COND = 128
B, T = 16, 16384
NCORES = 8
BL = B // NCORES          # samples per core
PAD = (K - 1) * DIL       # causal left pad = 8
NT = 512                  # matmul free-dim (one PSUM bank of fp32)
UW = 1024                 # processing unit width (2 PSUM banks)
NJ = T // UW
F = K * CH * 2 * CH       # 24576 adapter columns
FI = 2 * CH * K           # 384 adapter columns per input-channel row
XCHUNK = 4096             # x column chunk per input DMA

NCHUNK = 8                # aw DMA chunks
CW = F // NCHUNK          # 3072 adapter columns per chunk
AWLINES = 32              # DMA lines per aw chunk (4 partitions each)
XLINES = 32               # DMA lines per x chunk (4 partitions each)

F32 = mybir.dt.float32
F32R = mybir.dt.float32r
BF16 = mybir.dt.bfloat16
AF = mybir.ActivationFunctionType
ALU = mybir.AluOpType

# Set by test.py to capture a profile; harness path leaves these alone.
TRACE = False
LAST_RESULTS = None

_NC = None


def _build():
    nc = bacc.Bacc("TRN2", target_bir_lowering=False, debug=False)

    # x and aw arrive in "fat line" layouts (few lines x large contiguous
    # bytes): per-queue DMA descriptor throughput is what limits the input
    # load, and 24-32KB descriptors run ~2x faster than 6KB ones.
    x2_d = nc.dram_tensor(
        "x2", [T // XCHUNK, XLINES, (128 // XLINES) * XCHUNK], BF16,
        kind="ExternalInput",
    )
    cT_d = nc.dram_tensor("cT", [COND, BL], BF16, kind="ExternalInput")
    cTf_d = nc.dram_tensor("cTf", [COND, BL], F32R, kind="ExternalInput")
    aw_d = nc.dram_tensor(
        "aw_r", [NCHUNK, AWLINES, (COND // AWLINES) * CW], BF16,
        kind="ExternalInput",
    )
    ab_d = nc.dram_tensor("ab_r", [2 * CH, FI], F32R, kind="ExternalInput")
    bw_d = nc.dram_tensor("bw", [COND, 2 * CH], F32R, kind="ExternalInput")
    bb_d = nc.dram_tensor("bb", [1, 2 * CH], F32R, kind="ExternalInput")
    riw2_d = nc.dram_tensor("riw2", [2 * CH, 2 * CH], BF16, kind="ExternalInput")
    rb2_d = nc.dram_tensor("rb2", [2 * CH, 1], F32, kind="ExternalInput")
    z_d = nc.dram_tensor("z_d", [BL, CH, T], BF16, kind="ExternalOutput")
    out_d = nc.dram_tensor("out_d", [BL, CH, T], BF16, kind="ExternalOutput")

    with tile.TileContext(nc) as tc:
        with (
            tc.tile_pool(name="const", bufs=1) as constp,
            tc.tile_pool(name="xpool", bufs=1) as xpool,
            tc.tile_pool(name="kern", bufs=1) as kernp,
        ):
            # Both samples stacked on 128 partitions, left-padded by PAD.
            xbuf = xpool.tile([2 * CH, PAD + T], BF16)
            nc.vector.memset(xbuf[:, 0:PAD], 0.0)
            # (x column chunks are DMA'd inside the phase-A loop below,
            # gated on adapter progress.)

            # Warm the PE p-state during the input load: back-to-back dummy
            # matmuls keep the tensor engine continuously busy so the
            # adapter matmuls run at full clock.
            warm_w = constp.tile([2 * CH, 2 * CH], BF16)
            nc.vector.memset(warm_w[:, :], 0.0)
            warm_x = constp.tile([2 * CH, NT], BF16)
            nc.vector.memset(warm_x[:, :], 0.0)
            # Touch Sigmoid (then Tanh) once so the activation table that
            # serves both is loaded before phase B.
            warm_a = constp.tile([1, 2], F32)
            nc.scalar.activation(warm_a[:, 0:1], warm_w[0:1, 0:1], AF.Sigmoid)
            nc.scalar.activation(warm_a[:, 1:2], warm_w[0:1, 0:1], AF.Tanh)

            # Small constants go via the GpSimd (SWDGE) queue: each
            # dma_start costs ~600ns of descriptor generation, and on the
            # Sync sequencer that would delay the adapter-weight chunks.
            cT_sb = constp.tile([COND, BL], BF16)
            nc.gpsimd.dma_start(cT_sb[:, :], cT_d[:, :])
            cTf_sb = constp.tile([COND, BL], F32R)
            nc.gpsimd.dma_start(cTf_sb[:, :], cTf_d[:, :])
            bw_sb = constp.tile([COND, 2 * CH], F32R)
            nc.gpsimd.dma_start(bw_sb[:, :], bw_d[:, :])
            bb_sb = constp.tile([1, 2 * CH], F32R)
            nc.gpsimd.dma_start(bb_sb[:, :], bb_d[:, :])
            riw2_sb = constp.tile([2 * CH, 2 * CH], BF16)
            nc.gpsimd.dma_start(riw2_sb[:, :], riw2_d[:, :])
            rb2_sb = constp.tile([2 * CH, 1], F32)
            nc.gpsimd.dma_start(rb2_sb[:, :], rb2_d[:, :])
            ab_sb = constp.tile([2 * CH, FI], F32R)
            nc.gpsimd.dma_start(ab_sb[:, :], ab_d[:, :])
            ones_sb = constp.tile([1, BL], F32R)
            nc.vector.memset(ones_sb[:, :].bitcast(F32), 1.0)

            kern_raw = kernp.tile([2 * CH, FI], BF16, name="kern_raw")
            bias_sb = constp.tile([2 * CH, BL], F32)
            bias2t = constp.tile([2 * CH, 1], F32)
            bias2s = constp.tile([2 * CH, 1], F32)
            # Block-diagonal per-tap weights: rows (i of s0; i of s1), cols
            # (o-half of s0; o-half of s1). Off-diagonal quadrants stay zero.
            kdt = [kernp.tile([2 * CH, 2 * CH], BF16, name=f"kdt{k}") for k in range(K)]
            kds = [kernp.tile([2 * CH, 2 * CH], BF16, name=f"kds{k}") for k in range(K)]
            for k in range(K):
                nc.vector.memset(kdt[k][:, :], 0.0)
                nc.vector.memset(kds[k][:, :], 0.0)

            # ---------------- phase A: adapter + conditioned bias ----------
            # 8 aw chunks of 3072 cols (8 input-channel rows each). Each
            # chunk's DMA is chained on the previous chunk's ARRIVAL via a
            # 1-element GpSimd seed copy — concurrently-issued DMAs
            # round-robin on the queues and would all finish together. The
            # x column chunks are released near the end of the aw stream.
            with (
                tc.tile_pool(name="awp", bufs=2) as awp,
                tc.tile_pool(name="apsum", bufs=6, space="PSUM") as apsum,
                tc.tile_pool(name="stg", bufs=3) as stgp,
                tc.tile_pool(name="bpsum", bufs=1, space="PSUM") as bpsum,
                tc.tile_pool(name="wpsum", bufs=1, space="PSUM") as wpsum,
            ):
                pw = wpsum.tile([2 * CH, NT], F32)
                for _ in range(8):
                    nc.tensor.matmul(
                        pw[:, :], warm_w[:, :], warm_x[:, :], start=True, stop=True
                    )

                prev_awt = None
                for c in range(NCHUNK):
                    awt = awp.tile([COND, CW], BF16, tag="aw")
                    if prev_awt is not None:
                        nc.gpsimd.tensor_copy(awt[0:1, 0:1], prev_awt[0:1, 0:1])
                    nc.sync.dma_start(awt[:, :], aw_d[c])
                    prev_awt = awt
                    stg = stgp.tile([BL, CW], BF16, tag="stg")
                    for u in range(CW // FI):
                        ps = apsum.tile([BL, FI], F32, tag="ap")
                        nc.tensor.matmul(
                            ps[:, :],
                            cT_sb[:, :],
                            awt[:, u * FI : (u + 1) * FI],
                            start=True,
                            stop=True,
                        )
                        if u % 2 == 0:
                            nc.scalar.activation(
                                stg[:, u * FI : (u + 1) * FI], ps[:, :], AF.Copy
                            )
                        else:
                            nc.vector.tensor_copy(
                                stg[:, u * FI : (u + 1) * FI], ps[:, :]
                            )
                    for s in range(BL):
                        nc.sync.dma_start(
                            kern_raw[CH * s + 8 * c : CH * s + 8 * c + 8, :],
                            stg[s : s + 1, :],
                        )
                    # Release the x chunks near the END of the aw stream:
                    # earlier release would round-robin with the remaining
                    # adapter weights and delay them; released at chunks
                    # 4..7 they land just ahead of their first conv reader.
                    if c >= 4:
                        cc = c - 4
                        nc.scalar.activation(
                            xbuf[0:1, PAD + cc * XCHUNK : PAD + cc * XCHUNK + 1],
                            stg[0:1, 0:1],
                            AF.Copy,
                        )
                        nc.sync.dma_start(
                            xbuf[:, PAD + cc * XCHUNK : PAD + (cc + 1) * XCHUNK],
                            x2_d[cc],
                        )

                # Conditioned bias (needed first by the j=0 activations).
                pb = bpsum.tile([2 * CH, BL], F32)
                nc.tensor.matmul(
                    pb[:, :], bw_sb[:, :], cTf_sb[:, :], start=True, stop=False
                )
                nc.tensor.matmul(
                    pb[:, :], bb_sb[:, :], ones_sb[:, :], start=False, stop=True
                )
                nc.vector.tensor_copy(bias_sb[:, :], pb[:, :])
                # Stacked per-half biases: bias2t[64*s + c] = bias[c, s].
                for s in range(BL):
                    nc.gpsimd.dma_start(
                        bias2t[CH * s : CH * (s + 1), 0:1], bias_sb[0:CH, s : s + 1]
                    )
                    nc.gpsimd.dma_start(
                        bias2s[CH * s : CH * (s + 1), 0:1],
                        bias_sb[CH : 2 * CH, s : s + 1],
                    )
                # Fill the diagonal blocks of each tap/half weight tile
                # directly on the vector engine (adds adapter_b and converts
                # to bf16 in the same pass; off-diagonal quadrants stay 0).
                for k in range(K):
                    nc.vector.tensor_add(
                        kdt[k][0:CH, 0:CH],
                        kern_raw[0:CH, 128 * k : 128 * k + CH],
                        ab_sb[0:CH, 128 * k : 128 * k + CH],
                    )
                    nc.vector.tensor_add(
                        kdt[k][CH:, CH:],
                        kern_raw[CH:, 128 * k : 128 * k + CH],
                        ab_sb[CH:, 128 * k : 128 * k + CH],
                    )
                    nc.vector.tensor_add(
                        kds[k][0:CH, 0:CH],
                        kern_raw[0:CH, 128 * k + CH : 128 * (k + 1)],
                        ab_sb[0:CH, 128 * k + CH : 128 * (k + 1)],
                    )
                    nc.vector.tensor_add(
                        kds[k][CH:, CH:],
                        kern_raw[CH:, 128 * k + CH : 128 * (k + 1)],
                        ab_sb[CH:, 128 * k + CH : 128 * (k + 1)],
                    )
                # Bridge the PE p-state through the kd-add window: dummy
                # matmuls that read kern_raw are dependency-placed right
                # after the adapter finishes.
                for _ in range(6):
                    nc.tensor.matmul(
                        pw[:, 0:FI],
                        warm_w[:, :],
                        kern_raw[:, :],
                        start=True,
                        stop=True,
                    )

            # ---------------- phase B: conv + gate + residual --------------
            with (
                tc.tile_pool(name="ypsum", bufs=2, space="PSUM") as ypsum,
                tc.tile_pool(name="work", bufs=3) as workp,
            ):
                for j in range(NJ):
                    pyt = ypsum.tile([2 * CH, UW], F32, tag="pyt")
                    pys = ypsum.tile([2 * CH, UW], F32, tag="pys")
                    # k outer / h inner so each weight tile is loaded once.
                    for k in range(K):
                        for h in range(UW // NT):
                            c0 = j * UW + h * NT + DIL * k
                            nc.tensor.matmul(
                                pyt[:, h * NT : (h + 1) * NT],
                                kdt[k][:, :],
                                xbuf[:, c0 : c0 + NT],
                                start=(k == 0),
                                stop=(k == K - 1),
                            )
                    for k in range(K):
                        for h in range(UW // NT):
                            c0 = j * UW + h * NT + DIL * k
                            nc.tensor.matmul(
                                pys[:, h * NT : (h + 1) * NT],
                                kds[k][:, :],
                                xbuf[:, c0 : c0 + NT],
                                start=(k == 0),
                                stop=(k == K - 1),
                            )
                    th = workp.tile([2 * CH, UW], BF16, tag="th")
                    nc.scalar.activation(
                        th[:, :], pyt[:, :], AF.Tanh, bias=bias2t[:, 0:1]
                    )
                    sg = workp.tile([2 * CH, UW], BF16, tag="sg")
                    nc.scalar.activation(
                        sg[:, :], pys[:, :], AF.Sigmoid, bias=bias2s[:, 0:1]
                    )
                    z2 = workp.tile([2 * CH, UW], BF16, tag="z2")
                    nc.vector.tensor_mul(z2[:, :], th[:, :], sg[:, :])
                    for s in range(BL):
                        nc.sync.dma_start(
                            z_d[s][:, j * UW : (j + 1) * UW],
                            z2[CH * s : CH * (s + 1), :],
                        )
                    # Residual matmul reuses pyt's PSUM banks (WAR on tanh).
                    for h in range(UW // NT):
                        nc.tensor.matmul(
                            pyt[:, h * NT : (h + 1) * NT],
                            riw2_sb[:, :],
                            z2[:, h * NT : (h + 1) * NT],
                            start=True,
                            stop=True,
                        )
                    ot = workp.tile([2 * CH, UW], BF16, tag="ot")
                    nc.vector.scalar_tensor_tensor(
                        ot[:, :],
                        pyt[:, :],
                        rb2_sb[:, 0:1],
                        xbuf[:, PAD + j * UW : PAD + (j + 1) * UW],
                        ALU.add,
                        ALU.add,
                    )
                    # out stores go via the (otherwise idle) GpSimd queue so
                    # the Sync sequencer only issues the z stores.
                    for s in range(BL):
                        nc.gpsimd.dma_start(
                            out_d[s][:, j * UW : (j + 1) * UW],
                            ot[CH * s : CH * (s + 1), :],
                        )

    nc.compile()
    return nc


def get_nc():
    global _NC
    if _NC is None:
        _NC = _build()
    return _NC


def make_in_maps(inputs):
    import ml_dtypes

    BF = ml_dtypes.bfloat16

    x = np.asarray(inputs["x"], np.float32)
    c = np.asarray(inputs["c"], np.float32)
    aw = np.asarray(inputs["adapter_w"], np.float32)
    ab = np.asarray(inputs["adapter_b"], np.float32)
    bw = np.ascontiguousarray(np.asarray(inputs["bias_w"], np.float32))
    bb = np.asarray(inputs["bias_b"], np.float32).reshape(1, 2 * CH)
    rw = np.asarray(inputs["resi_w"], np.float32)
    rb = np.asarray(inputs["resi_b"], np.float32).reshape(CH, 1)

    # adapter columns [cond, (o,i,k)] -> [cond, (i,k,o)], then chunked into
    # fat DMA lines: [chunk, 32 lines, 4 cond-rows x CW cols].
    aw_r = aw.reshape(COND, 2 * CH, CH, K).transpose(0, 2, 3, 1).reshape(COND, F)
    aw_r = np.ascontiguousarray(
        np.stack(
            [
                aw_r[:, c * CW : (c + 1) * CW].reshape(AWLINES, -1)
                for c in range(NCHUNK)
            ]
        ).astype(BF)
    )
    ab_r1 = ab.reshape(2 * CH, CH, K).transpose(1, 2, 0).reshape(CH, FI)
    ab_r = np.ascontiguousarray(np.concatenate([ab_r1, ab_r1], axis=0))
    riw2 = np.zeros((2 * CH, 2 * CH), np.float32)
    riw2[0:CH, 0:CH] = rw.T
    riw2[CH:, CH:] = rw.T
    riw2 = np.ascontiguousarray(riw2.astype(BF))
    rb2 = np.ascontiguousarray(np.concatenate([rb, rb], axis=0))
    x_bf = x.astype(BF)

    in_maps = []
    for m in range(NCORES):
        sl = slice(BL * m, BL * (m + 1))
        x2f = x_bf[sl].reshape(2 * CH, T)
        x2c = np.ascontiguousarray(
            np.stack(
                [
                    x2f[:, cc * XCHUNK : (cc + 1) * XCHUNK].reshape(XLINES, -1)
                    for cc in range(T // XCHUNK)
                ]
            )
        )
        in_maps.append(
            {
                "x2": x2c,
                "cT": np.ascontiguousarray(c[sl].T.astype(BF)),
                "cTf": np.ascontiguousarray(c[sl].T),
                "aw_r": aw_r,
                "ab_r": ab_r,
                "bw": bw,
                "bb": bb,
                "riw2": riw2,
                "rb2": rb2,
            }
        )
    return in_maps


def kernel(**inputs):
    global LAST_RESULTS
    nc = get_nc()
    in_maps = make_in_maps(inputs)
    res = run_bass_kernel_spmd(nc, in_maps, list(range(NCORES)), trace=TRACE)
    LAST_RESULTS = res
    out = np.empty((B, CH, T), np.float32)
    z = np.empty((B, CH, T), np.float32)
    for m in range(NCORES):
        out[BL * m : BL * (m + 1)] = np.asarray(
            res.results[m]["out_d"], dtype=np.float32
        )
        z[BL * m : BL * (m + 1)] = np.asarray(res.results[m]["z_d"], dtype=np.float32)
    return out, z


# revision 34
# speedup vs baseline: 1.3427x; 1.3427x over previous
"""Gated TCN layer (fully conditioned) as a Bass/Tile kernel on 8 NeuronCores.

Reference computation (per sample b):
    kern = (c @ adapter_w + adapter_b).reshape(2*CH, CH, K)
    y    = dilated causal conv of x with per-sample kern (K=3, dil=4)
    y   += (c @ bias_w + bias_b)[:, None]
    z    = tanh(y[:CH]) * sigmoid(y[CH:])
    out  = resi_w @ z + resi_b + x
Returns (out, z).

Sharding: data-parallel over batch, 2 samples per core. Both samples are
stacked on the 128 SBUF partitions (rows 0-63 = sample 0, 64-127 = sample 1)
with block-diagonal per-tap dynamic kernels, so every matmul contracts over
128 partitions and every activation / vector op runs at full 128-partition
width. All heavy traffic and matmuls are bf16 (outputs are converted back to
f32 on the host); PSUM accumulation stays f32.
"""

import numpy as np

from concourse import bacc, mybir, tile
from concourse.bass_utils import run_bass_kernel_spmd

K = 3
DIL = 4
CH = 64
COND = 128
B, T = 16, 16384
NCORES = 8
BL = B // NCORES          # samples per core
PAD = (K - 1) * DIL       # causal left pad = 8
NT = 512                  # matmul free-dim (one PSUM bank of fp32)
UW = 1024                 # processing unit width (2 PSUM banks)
NJ = T // UW
F = K * CH * 2 * CH       # 24576 adapter columns
FI = 2 * CH * K           # 384 adapter columns per input-channel row
XCHUNK = 4096             # x column chunk per input DMA

NPIECE = 16               # aw DMA pieces (issued in order, ungated)
PW = F // NPIECE          # 1536 adapter columns per piece
NSG = 4                   # scatter groups (4 pieces each)

F32 = mybir.dt.float32
F32R = mybir.dt.float32r
BF16 = mybir.dt.bfloat16
AF = mybir.ActivationFunctionType
ALU = mybir.AluOpType

# Set by test.py to capture a profile; harness path leaves these alone.
TRACE = False
LAST_RESULTS = None

_NC = None


def _build():
    nc = bacc.Bacc("TRN2", target_bir_lowering=False, debug=False)

    x2_d = nc.dram_tensor("x2", [2 * CH, T], BF16, kind="ExternalInput")
    cT_d = nc.dram_tensor("cT", [COND, BL], BF16, kind="ExternalInput")
    cTf_d = nc.dram_tensor("cTf", [COND, BL], F32R, kind="ExternalInput")
    aw_d = nc.dram_tensor("aw_r", [COND, F], BF16, kind="ExternalInput")
    ab_d = nc.dram_tensor("ab_r", [2 * CH, FI], F32R, kind="ExternalInput")
    bw_d = nc.dram_tensor("bw", [COND, 2 * CH], F32R, kind="ExternalInput")
    bb_d = nc.dram_tensor("bb", [1, 2 * CH], F32R, kind="ExternalInput")
    riw2_d = nc.dram_tensor("riw2", [2 * CH, 2 * CH], BF16, kind="ExternalInput")
    rb2_d = nc.dram_tensor("rb2", [2 * CH, 1], F32, kind="ExternalInput")
    z_d = nc.dram_tensor("z_d", [BL, CH, T], BF16, kind="ExternalOutput")
    out_d = nc.dram_tensor("out_d", [BL, CH, T], BF16, kind="ExternalOutput")

    with tile.TileContext(nc) as tc:
        with (
            tc.tile_pool(name="const", bufs=1) as constp,
            tc.tile_pool(name="xpool", bufs=1) as xpool,
            tc.tile_pool(name="kern", bufs=1) as kernp,
        ):
            # Both samples stacked on 128 partitions, left-padded by PAD.
            xbuf = xpool.tile([2 * CH, PAD + T], BF16)
            nc.vector.memset(xbuf[:, 0:PAD], 0.0)
            # (x column chunks are DMA'd inside the phase-A loop below,
            # gated on adapter progress.)

            # Warm the PE p-state during the input load: back-to-back dummy
            # matmuls keep the tensor engine continuously busy so the
            # adapter matmuls run at full clock.
            warm_w = constp.tile([2 * CH, 2 * CH], BF16)
            nc.vector.memset(warm_w[:, :], 0.0)
            warm_x = constp.tile([2 * CH, NT], BF16)
            nc.vector.memset(warm_x[:, :], 0.0)
            # Touch Sigmoid (then Tanh) once so the activation table that
            # serves both is loaded before phase B.
            warm_a = constp.tile([1, 2], F32)
            nc.scalar.activation(warm_a[:, 0:1], warm_w[0:1, 0:1], AF.Sigmoid)
            nc.scalar.activation(warm_a[:, 1:2], warm_w[0:1, 0:1], AF.Tanh)

            # Small constants go via the GpSimd (SWDGE) queue: each
            # dma_start costs ~600ns of descriptor generation, and on the
            # Sync sequencer that would delay the adapter-weight chunks.
            cT_sb = constp.tile([COND, BL], BF16)
            nc.gpsimd.dma_start(cT_sb[:, :], cT_d[:, :])
            cTf_sb = constp.tile([COND, BL], F32R)
            nc.gpsimd.dma_start(cTf_sb[:, :], cTf_d[:, :])
            bw_sb = constp.tile([COND, 2 * CH], F32R)
            nc.gpsimd.dma_start(bw_sb[:, :], bw_d[:, :])
            bb_sb = constp.tile([1, 2 * CH], F32R)
            nc.gpsimd.dma_start(bb_sb[:, :], bb_d[:, :])
            riw2_sb = constp.tile([2 * CH, 2 * CH], BF16)
            nc.gpsimd.dma_start(riw2_sb[:, :], riw2_d[:, :])
            rb2_sb = constp.tile([2 * CH, 1], F32)
            nc.gpsimd.dma_start(rb2_sb[:, :], rb2_d[:, :])
            ab_sb = constp.tile([2 * CH, FI], F32R)
            nc.gpsimd.dma_start(ab_sb[:, :], ab_d[:, :])
            ones_sb = constp.tile([1, BL], F32R)
            nc.vector.memset(ones_sb[:, :].bitcast(F32), 1.0)

            kern_raw = kernp.tile([2 * CH, FI], BF16, name="kern_raw")
            bias_sb = constp.tile([2 * CH, BL], F32)
            bias2t = constp.tile([2 * CH, 1], F32)
            bias2s = constp.tile([2 * CH, 1], F32)
            # Block-diagonal per-tap weights: rows (i of s0; i of s1), cols
            # (o-half of s0; o-half of s1). Off-diagonal quadrants stay zero.
            kdt = [kernp.tile([2 * CH, 2 * CH], BF16, name=f"kdt{k}") for k in range(K)]
            kds = [kernp.tile([2 * CH, 2 * CH], BF16, name=f"kds{k}") for k in range(K)]
            for k in range(K):
                nc.vector.memset(kdt[k][:, :], 0.0)
                nc.vector.memset(kds[k][:, :], 0.0)

            # ---------------- phase A: adapter + conditioned bias ----------
            # Input DMA strategy: queues drain FIFO, so issue everything
            # ungated in priority order — 16 small aw pieces first (into one
            # persistent tile; the tile framework tracks column-range deps),
            # then the x chunks. This keeps the queues deeply fed (high
            # aggregate bandwidth) while early pieces still finish early.
            awt = kernp.tile([COND, F], BF16, name="awt")
            for p in range(NPIECE):
                nc.sync.dma_start(
                    awt[:, p * PW : (p + 1) * PW], aw_d[:, p * PW : (p + 1) * PW]
                )
            for cc in range(T // XCHUNK):
                nc.sync.dma_start(
                    xbuf[:, PAD + cc * XCHUNK : PAD + (cc + 1) * XCHUNK],
                    x2_d[:, cc * XCHUNK : (cc + 1) * XCHUNK],
                )

            with (
                tc.tile_pool(name="apsum", bufs=2, space="PSUM") as apsum,
                tc.tile_pool(name="stg", bufs=2) as stgp,
                tc.tile_pool(name="bpsum", bufs=1, space="PSUM") as bpsum,
                tc.tile_pool(name="wpsum", bufs=1, space="PSUM") as wpsum,
            ):
                pw = wpsum.tile([2 * CH, NT], F32)
                for _ in range(8):
                    nc.tensor.matmul(
                        pw[:, :], warm_w[:, :], warm_x[:, :], start=True, stop=True
                    )

                # 4 scatter groups of 4 pieces; within a group each piece
                # gets 3 matmuls of 512 and ONE batched [2, 1536] copy
                # (alternating scalar/vector) into the staging tile.
                GW = F // NSG  # 6144 cols per scatter group
                for q in range(NSG):
                    stg = stgp.tile([BL, GW], BF16, tag="stg")
                    for v in range(NPIECE // NSG):
                        p = q * (NPIECE // NSG) + v
                        ps = apsum.tile([BL, PW], F32, tag="ap")
                        for w in range(PW // NT):
                            nc.tensor.matmul(
                                ps[:, w * NT : (w + 1) * NT],
                                cT_sb[:, :],
                                awt[:, p * PW + w * NT : p * PW + (w + 1) * NT],
                                start=True,
                                stop=True,
                            )
                        if p % 2 == 0:
                            nc.scalar.activation(
                                stg[:, v * PW : (v + 1) * PW], ps[:, :], AF.Copy
                            )
                        else:
                            nc.vector.tensor_copy(
                                stg[:, v * PW : (v + 1) * PW], ps[:, :]
                            )
                    for s in range(BL):
                        nc.gpsimd.dma_start(
                            kern_raw[CH * s + 16 * q : CH * s + 16 * q + 16, :],
                            stg[s : s + 1, :],
                        )

                # Conditioned bias (needed first by the j=0 activations).
                pb = bpsum.tile([2 * CH, BL], F32)
                nc.tensor.matmul(
                    pb[:, :], bw_sb[:, :], cTf_sb[:, :], start=True, stop=False
                )
                nc.tensor.matmul(
                    pb[:, :], bb_sb[:, :], ones_sb[:, :], start=False, stop=True
                )
                nc.vector.tensor_copy(bias_sb[:, :], pb[:, :])
                # Stacked per-half biases: bias2t[64*s + c] = bias[c, s].
                for s in range(BL):
                    nc.gpsimd.dma_start(
                        bias2t[CH * s : CH * (s + 1), 0:1], bias_sb[0:CH, s : s + 1]
                    )
                    nc.gpsimd.dma_start(
                        bias2s[CH * s : CH * (s + 1), 0:1],
                        bias_sb[CH : 2 * CH, s : s + 1],
                    )
                # Fill the diagonal blocks of each tap/half weight tile
                # directly on the vector engine (adds adapter_b and converts
                # to bf16 in the same pass; off-diagonal quadrants stay 0).
                for k in range(K):
                    nc.vector.tensor_add(
                        kdt[k][0:CH, 0:CH],
                        kern_raw[0:CH, 128 * k : 128 * k + CH],
                        ab_sb[0:CH, 128 * k : 128 * k + CH],
                    )
                    nc.vector.tensor_add(
                        kdt[k][CH:, CH:],
                        kern_raw[CH:, 128 * k : 128 * k + CH],
                        ab_sb[CH:, 128 * k : 128 * k + CH],
                    )
                    nc.vector.tensor_add(
                        kds[k][0:CH, 0:CH],
                        kern_raw[0:CH, 128 * k + CH : 128 * (k + 1)],
                        ab_sb[0:CH, 128 * k + CH : 128 * (k + 1)],
                    )
                    nc.vector.tensor_add(
                        kds[k][CH:, CH:],
                        kern_raw[CH:, 128 * k + CH : 128 * (k + 1)],
                        ab_sb[CH:, 128 * k + CH : 128 * (k + 1)],
                    )
                # Bridge the PE p-state through the kd-add window: dummy
                # matmuls that read kern_raw are dependency-placed right
                # after the adapter finishes.
                for _ in range(6):
                    nc.tensor.matmul(
                        pw[:, 0:FI],
                        warm_w[:, :],
                        kern_raw[:, :],
                        start=True,
                        stop=True,
                    )

            # ---------------- phase B: conv + gate + residual --------------
            with (
                tc.tile_pool(name="ypsum", bufs=2, space="PSUM") as ypsum,
                tc.tile_pool(name="work", bufs=3) as workp,
            ):
                for j in range(NJ):
                    pyt = ypsum.tile([2 * CH, UW], F32, tag="pyt")
                    pys = ypsum.tile([2 * CH, UW], F32, tag="pys")
                    # k outer / h inner so each weight tile is loaded once.
                    for k in range(K):
                        for h in range(UW // NT):
                            c0 = j * UW + h * NT + DIL * k
                            nc.tensor.matmul(
                                pyt[:, h * NT : (h + 1) * NT],
                                kdt[k][:, :],
                                xbuf[:, c0 : c0 + NT],
                                start=(k == 0),
                                stop=(k == K - 1),
                            )
                    for k in range(K):
                        for h in range(UW // NT):
                            c0 = j * UW + h * NT + DIL * k
                            nc.tensor.matmul(
                                pys[:, h * NT : (h + 1) * NT],
                                kds[k][:, :],
                                xbuf[:, c0 : c0 + NT],
                                start=(k == 0),
                                stop=(k == K - 1),
                            )
                    th = workp.tile([2 * CH, UW], BF16, tag="th")
                    nc.scalar.activation(
                        th[:, :], pyt[:, :], AF.Tanh, bias=bias2t[:, 0:1]
                    )
                    sg = workp.tile([2 * CH, UW], BF16, tag="sg")
                    nc.scalar.activation(
                        sg[:, :], pys[:, :], AF.Sigmoid, bias=bias2s[:, 0:1]
                    )
                    z2 = workp.tile([2 * CH, UW], BF16, tag="z2")
                    nc.vector.tensor_mul(z2[:, :], th[:, :], sg[:, :])
                    for s in range(BL):
                        nc.sync.dma_start(
                            z_d[s][:, j * UW : (j + 1) * UW],
                            z2[CH * s : CH * (s + 1), :],
                        )
                    # Residual matmul reuses pyt's PSUM banks (WAR on tanh).
                    for h in range(UW // NT):
                        nc.tensor.matmul(
                            pyt[:, h * NT : (h + 1) * NT],
                            riw2_sb[:, :],
                            z2[:, h * NT : (h + 1) * NT],
                            start=True,
                            stop=True,
                        )
                    ot = workp.tile([2 * CH, UW], BF16, tag="ot")
                    nc.vector.scalar_tensor_tensor(
                        ot[:, :],
                        pyt[:, :],
                        rb2_sb[:, 0:1],
                        xbuf[:, PAD + j * UW : PAD + (j + 1) * UW],
                        ALU.add,
                        ALU.add,
                    )
                    # out stores go via the (otherwise idle) GpSimd queue so
                    # the Sync sequencer only issues the z stores.
                    for s in range(BL):
                        nc.gpsimd.dma_start(
                            out_d[s][:, j * UW : (j + 1) * UW],
                            ot[CH * s : CH * (s + 1), :],
                        )

    nc.compile()
    return nc


def get_nc():
    global _NC
    if _NC is None:
        _NC = _build()
    return _NC


def make_in_maps(inputs):
    import ml_dtypes

    BF = ml_dtypes.bfloat16

    x = np.asarray(inputs["x"], np.float32)
    c = np.asarray(inputs["c"], np.float32)
    aw = np.asarray(inputs["adapter_w"], np.float32)
    ab = np.asarray(inputs["adapter_b"], np.float32)
    bw = np.ascontiguousarray(np.asarray(inputs["bias_w"], np.float32))
    bb = np.asarray(inputs["bias_b"], np.float32).reshape(1, 2 * CH)
    rw = np.asarray(inputs["resi_w"], np.float32)
    rb = np.asarray(inputs["resi_b"], np.float32).reshape(CH, 1)

    # adapter columns [cond, (o,i,k)] -> [cond, (i,k,o)]
    aw_r = np.ascontiguousarray(
        aw.reshape(COND, 2 * CH, CH, K).transpose(0, 2, 3, 1).reshape(COND, F)
        .astype(BF)
    )
    ab_r1 = ab.reshape(2 * CH, CH, K).transpose(1, 2, 0).reshape(CH, FI)
    ab_r = np.ascontiguousarray(np.concatenate([ab_r1, ab_r1], axis=0))
    riw2 = np.zeros((2 * CH, 2 * CH), np.float32)
    riw2[0:CH, 0:CH] = rw.T
    riw2[CH:, CH:] = rw.T
    riw2 = np.ascontiguousarray(riw2.astype(BF))
    rb2 = np.ascontiguousarray(np.concatenate([rb, rb], axis=0))
    x_bf = x.astype(BF)

    in_maps = []
    for m in range(NCORES):
        sl = slice(BL * m, BL * (m + 1))
        in_maps.append(
            {
                "x2": np.ascontiguousarray(x_bf[sl].reshape(2 * CH, T)),
                "cT": np.ascontiguousarray(c[sl].T.astype(BF)),
                "cTf": np.ascontiguousarray(c[sl].T),
                "aw_r": aw_r,
                "ab_r": ab_r,
                "bw": bw,
                "bb": bb,
                "riw2": riw2,
                "rb2": rb2,
            }
        )
    return in_maps


def kernel(**inputs):
    global LAST_RESULTS
    nc = get_nc()
    in_maps = make_in_maps(inputs)
    res = run_bass_kernel_spmd(nc, in_maps, list(range(NCORES)), trace=TRACE)
    LAST_RESULTS = res
    out = np.empty((B, CH, T), np.float32)
    z = np.empty((B, CH, T), np.float32)
    for m in range(NCORES):
        out[BL * m : BL * (m + 1)] = np.asarray(
            res.results[m]["out_d"], dtype=np.float32
        )
        z[BL * m : BL * (m + 1)] = np.asarray(res.results[m]["z_d"], dtype=np.float32)
    return out, z
